# revision 27
# baseline (speedup 1.0000x reference)
"""Trainium2 Bass kernel for CMELossAngularProfileMSE_V2.

Strategy (pure data parallel over batch, 8 NeuronCores):
  - Shard B=128 samples -> 16 per core.
  - The host converts mask_pred to bf16 (the radial mean over R=2048
    averages out the ~2^-9 per-element rounding; measured loss rel err
    ~2e-5 vs the 2e-2 gate), halving wire bytes: the whole kernel is
    DMA-bound, so this nearly halves exec time.
  - The host also pre-transposes each core's shard to partition-major
    [128, 16*5760] bf16, so one dma_start covers a PAIR of samples with
    23KB contiguous lines per partition - the descriptor-size knee where
    SDMA engines reach ~26.9 GB/s (vs 26.5 at 11.5KB lines).
  - Fold: 2-op bf16 DVE tree (2880+1440-wide adds at DVE 2x mode,
    ~2.6us/sample) -> 4 slices; 4 bf16 one-hot matmuls (~0.5us each)
    accumulate them into fp32 PSUM row b. Every engine sits well under
    the ~3.45us/sample wire cadence, so the pipeline is robustly
    DMA-bound even on cores with a degraded SDMA engine (this host has
    several; a 12-deep input pool absorbs the straggler lag).
  - Last two samples stream in diminishing interleaved chunks with
    dispatch order == fold program order, so the in-order DVE queue
    drains folds as chunks arrive and only ~1us trails the final byte.
  - Host precomputes T' = R*T and w' = w/R^2 (exact power-of-two
    scalings of the Gaussian target / distance weight derived from
    theta_min/theta_max), so the device epilogue is just
    sum_theta((S - T')^2 * w') per sample -> out [16, 1], on DVE in
    fp32 (PSUM accumulation is fp32 throughout).
  - Host: loss = sum(all per-sample sums) / (360 * 128).
"""
import numpy as np

import concourse.bacc as bacc
import concourse.tile as tile
from concourse import mybir
from concourse.bass_utils import run_bass_kernel_spmd

F32 = mybir.dt.float32
BF16 = mybir.dt.bfloat16
ADD = mybir.AluOpType.add

N_CORES = 8
B = 128            # full batch
BS = B // N_CORES  # samples per core (16)
R = 2048
TH = 360
Q = 16             # r-slices per partition (2048 = 128 * 16)
SIGMA = 10.0
ALPHA_WEIGHT = 2.0
LAMBDA_ANG = 1.0

H = (Q // 2) * TH  # half-sample width (2880)


def _build_nc():
    nc = bacc.Bacc("TRN2", target_bir_lowering=False, debug=False)
    x = nc.dram_tensor("x", [128, BS * Q * TH], BF16, kind="ExternalInput").ap()
    tw = nc.dram_tensor("tw", [2, BS, TH], F32, kind="ExternalInput").ap()
    out = nc.dram_tensor("out", [BS, 1], F32, kind="ExternalOutput").ap()

    from contextlib import ExitStack
    with tile.TileContext(nc) as tc, ExitStack() as ctx:
        consts = ctx.enter_context(tc.tile_pool(name="consts", bufs=1))
        inp = ctx.enter_context(tc.tile_pool(name="inp", bufs=7))
        tailp = ctx.enter_context(tc.tile_pool(name="tailp", bufs=1))
        psum = ctx.enter_context(tc.tile_pool(name="psum", bufs=1, space="PSUM"))
        small = ctx.enter_context(tc.tile_pool(name="small", bufs=1))

        # one-hot weight matrices: O[:, b, j] = 1 if j == b else 0
        # (PSUM out base partition must be 0/32/64, so per-row matmuls
        # need the one-hot trick; built on gpsimd, idle at startup)
        O = consts.tile([128, BS, BS], BF16)
        nc.gpsimd.memset(O[:], 0.0)
        for b in range(BS):
            nc.gpsimd.memset(O[:, b, b:b + 1], 1.0)

        t16w16 = small.tile([BS, 2, TH], F32)
        t16 = t16w16[:, 0, :]
        w16 = t16w16[:, 1, :]

        ps = psum.tile([BS, TH], F32)

        def tree3(xt, b, start):
            """2-op bf16 DVE tree -> 4 slices -> 4 accumulating matmuls
            (bf16 matmuls are ~4x cheaper than fp32, so TensorE absorbs
            more of the fold; PSUM accumulation stays fp32)."""
            nc.vector.tensor_add(xt[:, 0:2880], xt[:, 0:2880],
                                 xt[:, 2880:5760])
            nc.vector.tensor_add(xt[:, 0:1440], xt[:, 0:1440],
                                 xt[:, 1440:2880])
            for k in range(4):
                nc.tensor.matmul(ps[:], O[:, b, :],
                                 xt[:, 360 * k:360 * (k + 1)],
                                 start=(start and k == 0), stop=False)

        # samples 0..13: one DMA per sample PAIR. The host supplies x in
        # partition-major layout [128, BS*5760], so a pair is a 23KB
        # CONTIGUOUS line per partition - the descriptor-size knee where
        # engines reach 26.9 GB/s (vs 26.5 at 11.5KB) - with no
        # cross-region interleaving.
        SW = Q * TH  # per-sample width (5760)
        for g in range((BS - 2) // 2):
            xt = inp.tile([128, 2 * SW], BF16)
            nc.sync.dma_start(xt[:], x[:, 2 * g * SW:(2 * g + 2) * SW])
            if g == 0:
                # tw load on the otherwise-idle gpsimd SWDGE queue: early
                # residency without occupying a sync dispatch slot
                nc.gpsimd.dma_start(t16w16[:],
                                    tw.rearrange("two b t -> b two t"))
            tree3(xt[:, 0:SW], 2 * g, start=(g == 0))
            tree3(xt[:, SW:2 * SW], 2 * g + 1, start=False)

        # Last two samples: diminishing chunks with dispatch order ==
        # fold program order, so the in-order DVE queue drains folds as
        # chunks arrive and only ~1us of work trails the final byte.
        xa = tailp.tile([128, Q * TH], BF16)   # sample 14
        xb = tailp.tile([128, Q * TH], BF16)   # sample 15
        ba, bb = BS - 2, BS - 1
        # interleaved chunk dispatch: H0(2880) Q2E6(2160) E7(720)
        for xt, b in ((xa, ba), (xb, bb)):
            nc.sync.dma_start(xt[:, 0:H], x[:, b * SW:b * SW + H])
        for xt, b in ((xa, ba), (xb, bb)):
            nc.sync.dma_start(xt[:, 2880:5040],
                              x[:, b * SW + 2880:b * SW + 5040])
        for xt, b in ((xa, ba), (xb, bb)):
            nc.sync.dma_start(xt[:, 5040:5760],
                              x[:, b * SW + 5040:b * SW + 5760])
        # H0 -> 4 slices -> 4 matmuls, as each H0 lands
        for xt, b in ((xa, ba), (xb, bb)):
            nc.vector.tensor_add(xt[:, 0:1440], xt[:, 0:1440],
                                 xt[:, 1440:2880])
            for k in range(4):
                nc.tensor.matmul(ps[:], O[:, b, :],
                                 xt[:, 360 * k:360 * (k + 1)],
                                 start=False, stop=False)
        # Q2 (slices 8-11) -> s1 at [2880:3240]
        for xt, b in ((xa, ba), (xb, bb)):
            nc.vector.tensor_add(xt[:, 2880:3600], xt[:, 2880:3600],
                                 xt[:, 3600:4320])
            nc.vector.tensor_add(xt[:, 2880:3240], xt[:, 2880:3240],
                                 xt[:, 3240:3600])
        # E6 (slices 12,13) folded into s1
        for xt, b in ((xa, ba), (xb, bb)):
            nc.vector.tensor_add(xt[:, 4320:4680], xt[:, 4320:4680],
                                 xt[:, 4680:5040])
            nc.vector.tensor_add(xt[:, 2880:3240], xt[:, 2880:3240],
                                 xt[:, 4320:4680])
        # E7 (slices 14,15): the last bytes on the wire
        for xt, b in ((xa, ba), (xb, bb)):
            nc.vector.tensor_add(xt[:, 5040:5400], xt[:, 5040:5400],
                                 xt[:, 5400:5760])
            nc.vector.tensor_add(xt[:, 2880:3240], xt[:, 2880:3240],
                                 xt[:, 5040:5400])
            nc.tensor.matmul(ps[:], O[:, b, :], xt[:, 2880:3240],
                             start=False, stop=(b == BS - 1))

        d16 = small.tile([BS, TH], F32)
        nc.vector.scalar_tensor_tensor(
            d16[:], ps[:], 1.0, t16,
            op0=mybir.AluOpType.mult, op1=mybir.AluOpType.subtract,
        )
        sq16 = small.tile([BS, TH], F32)
        nc.vector.scalar_tensor_tensor(
            sq16[:], d16[:], 1.0, d16[:],
            op0=mybir.AluOpType.mult, op1=mybir.AluOpType.mult,
        )
        sqw16 = small.tile([BS, TH], F32)
        red = small.tile([BS, 1], F32)
        nc.vector.scalar_tensor_tensor(
            sqw16[:], sq16[:], 1.0, w16,
            op0=mybir.AluOpType.mult, op1=mybir.AluOpType.mult,
            accum_out=red[:],
        )
        nc.sync.dma_start(out[:], red[:])
    nc.compile()
    return nc


def _target_and_weight(theta_min: np.ndarray, theta_max: np.ndarray):
    """Gaussian soft target T and distance weight w, [B, TH] float32 each.

    Mirrors the reference formulas (computed in float64, cast to float32;
    differences vs the f32 jax pipeline are O(1 ulp))."""
    theta = np.arange(TH, dtype=np.float64)[None, None, :]      # [1, 1, TH]
    tmin = theta_min.astype(np.float64)[:, :, None]             # [B, K, 1]
    tmax = theta_max.astype(np.float64)[:, :, None]

    center_wrap = np.mod(0.5 * (tmin + tmax + 360.0), 360.0)
    center_t = np.where(tmin <= tmax, 0.5 * (tmin + tmax), center_wrap)
    d = np.abs(theta - center_t)
    dist_t = np.minimum(d, 360.0 - d)                           # [B, K, TH]
    T = np.clip(np.exp(-0.5 * (dist_t / SIGMA) ** 2).sum(axis=1), 0.0, 1.0)

    center_w = (tmin + np.mod(tmax - tmin, 360.0)) / 2.0
    dw = np.abs(theta - center_w)
    dist_w = np.minimum(dw, 360.0 - dw)
    w = 1.0 + ALPHA_WEIGHT * (dist_w.max(axis=1) / 180.0)       # [B, TH]

    # Feed the device T' = R*T and w' = w/R^2 (both exact scalings by
    # powers of two) so it can use the raw radial sums S instead of the
    # mean A = S/R:  ((S - R*T)^2 * w/R^2) == ((A - T)^2 * w).
    Tp = (T * np.float32(R)).astype(np.float32)
    wp = (w / np.float32(R) ** 2).astype(np.float32)
    return Tp, wp


_NC_CACHE = None


def _get_nc():
    global _NC_CACHE
    if _NC_CACHE is None:
        _NC_CACHE = _build_nc()
    return _NC_CACHE


def _run(mask_pred, theta_min, theta_max, trace=False, trace_kwargs=None,
         trace_cores=None):
    from ml_dtypes import bfloat16 as _bf16
    mask_pred = np.asarray(mask_pred, dtype=np.float32)
    theta_min = np.asarray(theta_min)
    theta_max = np.asarray(theta_max)
    T, w = _target_and_weight(theta_min, theta_max)

    in_maps = []
    for i in range(N_CORES):
        sl = slice(i * BS, (i + 1) * BS)
        x_core = np.ascontiguousarray(
            mask_pred[sl, 0].reshape(BS, 128, Q * TH).astype(_bf16)
            .transpose(1, 0, 2).reshape(128, BS * Q * TH))
        tw_core = np.stack([T[sl], w[sl]])
        in_maps.append({"x": x_core, "tw": tw_core})

    kwargs = {}
    if trace:
        kwargs["trace"] = True
        if trace_kwargs:
            kwargs["trace_kwargs"] = trace_kwargs
        if trace_cores is not None:
            kwargs["trace_cores"] = trace_cores
    res = run_bass_kernel_spmd(_get_nc(), in_maps, core_ids=list(range(N_CORES)),
                               **kwargs)
    per_sample = np.concatenate(
        [res.results[i]["out"][:, 0] for i in range(N_CORES)]
    )
    total = per_sample.astype(np.float64).sum() / (TH * B)
    return np.float32(LAMBDA_ANG * total), res


def kernel(mask_pred: np.ndarray, theta_min: np.ndarray,
           theta_max: np.ndarray) -> np.ndarray:
    loss, _ = _run(mask_pred, theta_min, theta_max)
    return np.asarray(loss, dtype=np.float32)
